# revision 16
# baseline (speedup 1.0000x reference)
"""ChunkwiseRetentionMixer Trainium2 kernel.

Computes: out = rms_norm(cumsum(x @ Ws^T, axis=L)) @ Wo^T  for x (B,L,H)=(4,8192,1024),
Ws (64,1024), Wo (1024,64), all float32.

Sharding: 8 cores = (batch, L-half) pairs. Each core processes a [4096, 1024]
chunk. The cumsum carry into second-half chunks is seeded from a column-sum of
the first half of x (computed on host during sharding; it is 0.4% of the FLOPs
and removes the cross-core serial dependency).

Per-core dataflow (f32 end to end):
  x block [128l, 128h] --PE transpose--> xT [128h, 128l] (PSUM) --evac--> SBUF
  stateT [64s, 512l] = sum_h WsT_h.T @ xT_h      (PE, WsT stationary)
  cumT = tensor_tensor_scan(stateT, add)         (DVE, carry via `initial`)
  ms[128l, 1] = (cumT_block^2).T-reduce via PE   (lhsT = sq block, rhs = ones)
  rstd = 1/sqrt(ms/64 + eps)                     (ACT sqrt + DVE reciprocal)
  out block [128l, 1024h] = cumT_block.T @ WoT   (PE), evacuated PSUM->SBUF with
      the per-row rstd applied as the activation-copy scale (rms_norm scale
      commutes through the output projection).
"""

import os
import numpy as np

B, L, H, S = 4, 8192, 1024, 64
LC = L // 2            # rows per core chunk
P = 128                # partitions / block rows
NBLK = LC // P         # 32 l-blocks per core
GROUP_BLOCKS = 4       # l-blocks per pipeline group
GROUP_ROWS = GROUP_BLOCKS * P      # 512
NGROUPS = NBLK // GROUP_BLOCKS     # 8
KH = H // P            # 8 h-tiles of 128
EPS = 1.1920928955078125e-07

_CACHE = {}


def _emit(nc, tc, ctx, aps):
    import concourse.bass as bass
    from concourse import mybir

    f32 = mybir.dt.float32
    x_d, pf_d, ws_d, wo_d, id_d, out_d = (
        aps["x"], aps["pf"], aps["ws"], aps["wo"], aps["ident"], aps["out"])

    singles = ctx.enter_context(tc.tile_pool(name="singles", bufs=1))
    small = ctx.enter_context(tc.tile_pool(name="small", bufs=4))

    # ---- constants / weight prep -------------------------------------------
    ident = singles.tile([P, P], f32)
    nc.sync.dma_start(out=ident[:], in_=id_d[:, :])

    ws_sb = singles.tile([S, H], f32)
    nc.sync.dma_start(out=ws_sb[:], in_=ws_d[:, :])
    wo_sb = singles.tile([P, KH, S], f32)   # Wo rows tiled: [128, 8, 64]
    nc.sync.dma_start(
        out=wo_sb[:], in_=wo_d[:, :].rearrange("(k p) s -> p k s", p=P))
    pfT = singles.tile([P, KH], f32)        # prefix colsum, h on partitions
    nc.sync.dma_start(out=pfT[:], in_=pf_d[:, :])

    wsT = singles.tile([P, KH * S], f32)    # WsT: h-tile i at cols [64i, 64i+64)
    woT = singles.tile([S, H], f32)         # WoT [64s, 1024h]
    ones_col = singles.tile([S, 1], f32)
    nc.vector.memset(ones_col[:], 1.0)
    eps_col = singles.tile([P, 1], f32)
    nc.vector.memset(eps_col[:], EPS)

    c0 = small.tile([S, 1], f32)
    with tc.tile_pool(name="ps_prep", bufs=2, space="PSUM") as ps_prep:
        for i in range(KH):
            t_ps = ps_prep.tile([P, P], f32, tag="prep")
            nc.tensor.transpose(
                t_ps[:, 0:S], ws_sb[:, i * P:(i + 1) * P], ident[0:S, 0:S])
            nc.vector.tensor_copy(wsT[:, i * S:(i + 1) * S], t_ps[:, 0:S])
        for i in range(KH):
            t_ps = ps_prep.tile([P, P], f32, tag="prep")
            nc.tensor.transpose(t_ps[0:S, :], wo_sb[:, i, :], ident[:, :])
            nc.vector.tensor_copy(woT[:, i * P:(i + 1) * P], t_ps[0:S, :])

        # carry0 = pf @ Ws^T, shape [64, 1] (zero for first-half cores)
        c0_ps = ps_prep.tile([P, P], f32, tag="prep")
        for i in range(KH):
            nc.tensor.matmul(
                c0_ps[0:S, 0:1], wsT[:, i * S:(i + 1) * S], pfT[:, i:i + 1],
                start=(i == 0), stop=(i == KH - 1))
        nc.vector.tensor_copy(c0[:], c0_ps[0:S, 0:1])

    xin = ctx.enter_context(tc.tile_pool(name="xin", bufs=3))
    xtp = ctx.enter_context(tc.tile_pool(name="xtp", bufs=3))
    outp = ctx.enter_context(tc.tile_pool(name="outp", bufs=2))
    ps_xt = ctx.enter_context(tc.tile_pool(name="ps_xt", bufs=2, space="PSUM"))
    ps_st = ctx.enter_context(tc.tile_pool(name="ps_st", bufs=2, space="PSUM"))
    ps_ms = ctx.enter_context(tc.tile_pool(name="ps_ms", bufs=2, space="PSUM"))
    ps_out = ctx.enter_context(tc.tile_pool(name="ps_out", bufs=2, space="PSUM"))

    carry = c0
    carry_col = 0
    for g in range(NGROUPS):
        rows = slice(g * GROUP_ROWS, (g + 1) * GROUP_ROWS)
        x_sb = xin.tile([P, GROUP_BLOCKS, H], f32)
        nc.sync.dma_start(
            out=x_sb[:],
            in_=x_d[rows, :].rearrange("(n p) h -> p n h", p=P))

        # transpose x 128x128 tiles; xT_sb free = (n, i, l)
        xt_sb = xtp.tile([P, GROUP_BLOCKS * H], f32)
        for n in range(GROUP_BLOCKS):
            for half in range(2):
                xt_ps = ps_xt.tile([P, 4 * P], f32)
                for j in range(4):
                    i = half * 4 + j
                    nc.tensor.transpose(
                        xt_ps[:, j * P:(j + 1) * P],
                        x_sb[:, n, i * P:(i + 1) * P],
                        ident[:, :])
                eng = nc.vector if (n + half) % 2 == 0 else nc.scalar
                dst = xt_sb[:, n * H + half * 4 * P: n * H + (half + 1) * 4 * P]
                if eng is nc.vector:
                    nc.vector.tensor_copy(dst, xt_ps[:])
                else:
                    nc.scalar.activation(
                        dst, xt_ps[:], mybir.ActivationFunctionType.Copy)

        # projection: stateT [64, 512] accumulated over 8 h-tiles
        xt_v = xt_sb[:].rearrange(
            "p (n i l) -> p n i l", n=GROUP_BLOCKS, i=KH, l=P)
        st_ps = ps_st.tile([S, GROUP_ROWS], f32)
        for i in range(KH):
            nc.tensor.matmul(
                st_ps[:], wsT[:, i * S:(i + 1) * S], xt_v[:, :, i, :],
                start=(i == 0), stop=(i == KH - 1))
        st_sb = small.tile([S, GROUP_ROWS], f32)
        nc.vector.tensor_copy(st_sb[:], st_ps[:])

        # running cumsum along l with carry chaining
        cum_sb = small.tile([S, GROUP_ROWS], f32)
        nc.vector.tensor_tensor_scan(
            cum_sb[:], st_sb[:], st_sb[:], carry[:, carry_col:carry_col + 1],
            mybir.AluOpType.add, mybir.AluOpType.bypass)
        carry, carry_col = cum_sb, GROUP_ROWS - 1

        out_sb = outp.tile([P, GROUP_BLOCKS, H], f32)
        for n in range(GROUP_BLOCKS):
            blk = cum_sb[:, n * P:(n + 1) * P]          # [64, 128]
            sq = small.tile([S, P], f32)
            nc.vector.tensor_mul(sq[:], blk, blk)
            ms_ps = ps_ms.tile([P, 1], f32)
            nc.tensor.matmul(ms_ps[:], sq[:], ones_col[:], start=True, stop=True)
            rstd = small.tile([P, 1], f32)
            nc.scalar.activation(
                rstd[:], ms_ps[:], mybir.ActivationFunctionType.Sqrt,
                bias=eps_col[:], scale=1.0 / S)
            nc.vector.reciprocal(rstd[:], rstd[:])

            for m in range(2):
                o_ps = ps_out.tile([P, H // 2], f32)
                nc.tensor.matmul(
                    o_ps[:], blk, woT[:, m * (H // 2):(m + 1) * (H // 2)],
                    start=True, stop=True)
                nc.scalar.activation(
                    out_sb[:, n, m * (H // 2):(m + 1) * (H // 2)],
                    o_ps[:], mybir.ActivationFunctionType.Copy,
                    bias=0.0, scale=rstd[:])

        nc.sync.dma_start(
            out=out_d[rows, :].rearrange("(n p) h -> p n h", p=P),
            in_=out_sb[:])


def _build():
    if "nc" in _CACHE:
        return _CACHE["nc"]
    from contextlib import ExitStack
    import concourse.bacc as bacc
    import concourse.tile as tile
    from concourse import mybir

    f32 = mybir.dt.float32
    nc = bacc.Bacc("TRN2", target_bir_lowering=False, debug=False,
                   num_devices=8)
    aps = {
        "x": nc.dram_tensor("x", [LC, H], f32, kind="ExternalInput"),
        "pf": nc.dram_tensor("pf", [P, KH], f32, kind="ExternalInput"),
        "ws": nc.dram_tensor("ws", [S, H], f32, kind="ExternalInput"),
        "wo": nc.dram_tensor("wo", [H, S], f32, kind="ExternalInput"),
        "ident": nc.dram_tensor("ident", [P, P], f32, kind="ExternalInput"),
        "out": nc.dram_tensor("out", [LC, H], f32, kind="ExternalOutput"),
    }
    with tile.TileContext(nc) as tc:
        with ExitStack() as ctx:
            _emit(nc, tc, ctx, aps)
    nc.compile()
    _CACHE["nc"] = nc
    return nc


def kernel(x, Ws, Wo, _trace=False, _trace_kwargs=None):
    from concourse.bass_utils import run_bass_kernel_spmd

    nc = _build()
    x = np.ascontiguousarray(np.asarray(x, dtype=np.float32))
    ws = np.ascontiguousarray(np.asarray(Ws, dtype=np.float32))
    wo = np.ascontiguousarray(np.asarray(Wo, dtype=np.float32))
    ident = np.eye(P, dtype=np.float32)

    in_maps = []
    for c in range(8):
        b, half = c // 2, c % 2
        xc = np.ascontiguousarray(x[b, half * LC:(half + 1) * LC, :])
        if half:
            pf = x[b, :LC, :].sum(axis=0, dtype=np.float64).astype(np.float32)
        else:
            pf = np.zeros(H, dtype=np.float32)
        pfT = np.ascontiguousarray(pf.reshape(KH, P).T)
        in_maps.append({"x": xc, "pf": pfT, "ws": ws, "wo": wo, "ident": ident})

    res = run_bass_kernel_spmd(nc, in_maps, list(range(8)), trace=_trace,
                               **(_trace_kwargs or {}))
    kernel._last_result = res
    kernel._last_in_maps = in_maps
    out = np.empty((B, L, H), dtype=np.float32)
    for c in range(8):
        b, half = c // 2, c % 2
        out[b, half * LC:(half + 1) * LC, :] = res.results[c]["out"]
    return out
